# revision 53
# baseline (speedup 1.0000x reference)
"""Distributed attention kernel for one TRN2 chip (8 NeuronCores), v2.

Sharding: 16 heads / 8 cores = 2 heads per core (head-group parallel).
No collectives: each core computes a full [BT, C] PARTIAL of the output
projection from its 2 heads (contracting its 128-row slice of w_proj)
and the host sums the 8 bf16 partials (+ b_proj) during unshard. This
removes the AllGather sync stalls and the HAM cold-clock periods they
caused in v1.

Per core:
  - x arrives pre-transposed from host as [NCC, P, BT] bf16 c-chunks
  - QKV: Q^T,K^T head-dim-major; V token-major with a fused ones column
    per head (softmax denominator falls out of the PV matmul, rows 0-64
    of each head's accumulator)
  - attention as one flat software pipeline over 128 (unit, kc) steps,
    unit = (batch, 512-query chunk): dual row-tiled score matmuls
    (head0 on PE rows 0-63, head1 on rows 64-127 run concurrently),
    exp on ACT (the kernel bottleneck: ~1.2us per [128,1024] tile),
    PV accumulation; QKV(b1)/projection tasks are injected into PE
    slack between steps with dependency-aware pacing
  - proj partial: ot^T token chunks (stationary) x own w_proj rows ->
    [128 tok, 1024] fp32 -> bf16 -> DMA out
"""

import numpy as np

_CACHE = {}

P = 128
B, T, C = 2, 2048, 1024
BT = B * T
NCORE = 8
HD = 64  # head dim
CSL = 128  # per-core c-slice = 2 heads * 64
TQ = 512  # query chunk
NQC = T // TQ  # 4
KC = 128  # key chunk (partition dim)
NKC = T // KC  # 16
NCC = C // P  # 8 contraction chunks
NTC = BT // P  # 32 token chunks of 128
TB = T // P  # 16 token chunks per batch
NTH = 4  # x arrival regions
THL = BT // NTH  # 1024 tokens per region
VW = 128  # per-head V stationary: 64 v cols + 64 ones cols (denominator
# lands broadcast across PSUM rows 64-127 straight out of the PV matmul)


def _build():
    import concourse.bass as bass
    import concourse.tile as tile
    from concourse import bacc, mybir

    F32 = mybir.dt.float32
    BF16 = mybir.dt.bfloat16
    Exp = mybir.ActivationFunctionType.Exp

    nc = bacc.Bacc("TRN2", target_bir_lowering=False, debug=False, num_devices=NCORE)

    x_ext = nc.declare_dram_parameter("x", [NCC, P, BT], BF16, isOutput=False)
    wq_ext = nc.declare_dram_parameter("wq", [P, NCC, CSL], BF16, isOutput=False)
    wk_ext = nc.declare_dram_parameter("wk", [P, NCC, CSL], BF16, isOutput=False)
    wv_ext = nc.declare_dram_parameter("wv", [P, NCC, CSL], BF16, isOutput=False)
    wp_ext = nc.declare_dram_parameter("wp", [CSL, C], BF16, isOutput=False)
    bq_ext = nc.declare_dram_parameter("bq", [CSL, 1], F32, isOutput=False)
    bk_ext = nc.declare_dram_parameter("bk", [CSL, 1], F32, isOutput=False)
    bv_ext = nc.declare_dram_parameter("bv", [1, CSL], F32, isOutput=False)
    out_ext = nc.declare_dram_parameter("out", [BT, C], BF16, isOutput=True)

    with tile.TileContext(nc) as tc:
        with (
            nc.allow_low_precision("bf16 attention compute by design"),
            tc.tile_pool(name="pers", bufs=1) as pers,
            tc.tile_pool(name="stage", bufs=3) as stage,
            tc.tile_pool(name="ptp", bufs=4) as ptp,
            tc.tile_pool(name="psm", bufs=1, space="PSUM") as psm,
        ):
            # ---- persistent SBUF tiles ----
            xt_sb = pers.tile([P, NTH, NCC, THL], BF16, name="xt_sb")
            qt_sb = pers.tile([P, BT], BF16, name="qt_sb")  # Q^T (rows: 2*64 head dims)
            kt_sb = pers.tile([P, BT], BF16, name="kt_sb")
            v_sb = pers.tile([P, NTC, 2, VW], BF16, name="v_sb")  # [tok, chunk, head, vcol]
            ot_sb = pers.tile([P, BT], BF16, name="ot_sb")  # attention out^T (c-slice rows)
            wq_sb = pers.tile([P, NCC, CSL], BF16, name="wq_sb")
            wk_sb = pers.tile([P, NCC, CSL], BF16, name="wk_sb")
            wv_sb = pers.tile([P, NCC, CSL], BF16, name="wv_sb")
            wp_sb = pers.tile([P, C], BF16, name="wp_sb")  # own 128 rows of w_proj
            bq_sb = pers.tile([CSL, 1], F32, name="bq_sb")
            bk_sb = pers.tile([CSL, 1], F32, name="bk_sb")
            bv_row = pers.tile([1, CSL], F32, name="bv_row")
            bv_bc = pers.tile([P, CSL], F32, name="bv_bc")
            ones1 = pers.tile([1, TQ], F32, name="ones1")
            wsrc = pers.tile([P, TQ], BF16, name="wsrc")

            nc.gpsimd.memset(ones1[:], 1.0)
            nc.gpsimd.memset(wsrc[:], 1.0)

            # junk K=128 matmuls with no data deps: trip the HAM activity
            # monitor to K=8/8 (2.4GHz) during the input-DMA wait so the
            # first real matmuls don't run at the 1.2GHz cold clock (K=1
            # warmers don't register as PE activity). They write the op
            # accumulator ring, which is unused until the first PV.
            wt = psm.tile([P, 2, TQ], F32, tag="op", bufs=1, name="warm")
            for _ in range(16):
                nc.tensor.matmul(wt[:, 0, :], wsrc[:, 0:P], wsrc[:],
                                 start=True, stop=True)

            # preload the exp activation table (~2.7us) while DMAs run
            scr = stage.tile([1, 8], F32, tag="rc0", bufs=3, name="scr")
            nc.scalar.activation(scr[:], ones1[0:1, 0:8], Exp)

            # all inputs on the sync queue, smallest/most-urgent first (the
            # HW DGE rings fan out; gpsimd-triggered DMA is engine-driven
            # and slow, and the scalar queue is the ACT engine: ring
            # backpressure there would park exp() behind the triggers).
            # x is arrival-region major, th0 split in halves, so the first
            # K/Q projections unblock as early as possible.
            # denominator ones columns (gpsimd queue is otherwise idle at
            # startup: all input DMAs ride the sync queue)
            nc.gpsimd.memset(v_sb[:, :, :, HD:VW], 1.0)
            nc.sync.dma_start(bk_sb[:], bk_ext[:])
            nc.sync.dma_start(bq_sb[:], bq_ext[:])
            nc.sync.dma_start(bv_row[:], bv_ext[:])
            nc.sync.dma_start(wk_sb[:], wk_ext[:])
            nc.sync.dma_start(wq_sb[:], wq_ext[:])
            nc.sync.dma_start(wv_sb[:], wv_ext[:])
            for c in range(NCC):
                nc.sync.dma_start(xt_sb[:, 0, c, 0:TQ], x_ext[c, :, 0:TQ])
            for c in range(NCC):
                nc.sync.dma_start(xt_sb[:, 0, c, TQ:THL], x_ext[c, :, TQ:THL])
            nc.sync.dma_start(wp_sb[:], wp_ext[:])
            for th in range(1, NTH):
                for c in range(NCC):
                    nc.sync.dma_start(
                        xt_sb[:, th, c, :],
                        x_ext[c, :, th * THL:(th + 1) * THL],
                    )

            def xt(c, t0, n):
                th, off = divmod(t0, THL)
                assert off + n <= THL
                return xt_sb[:, th, c, off:off + n]

            qk_ps = {}

            def qk_half(w_sb, b_sb, dst, t8, half):
                # half a K/Q chunk (4 of 8 contraction MMs): injected tasks
                # must stay under ~1us of PE time or the scheduler parks
                # them ahead of the next S pair and the exp pipeline stalls
                key = (id(dst), t8)
                if half == 0:
                    qk_ps[key] = psm.tile([P, TQ], F32, tag="mm", bufs=2, name="ps_qk")
                for c in range(4 * half, 4 * half + 4):
                    nc.tensor.matmul(
                        qk_ps[key][:], w_sb[:, c, :], xt(c, t8 * TQ, TQ),
                        start=(c == 0), stop=(c == NCC - 1),
                    )
                if half == 1:
                    nc.vector.tensor_scalar_add(
                        dst[:, t8 * TQ:(t8 + 1) * TQ], qk_ps.pop(key)[:], b_sb[:])

            def v_chunk(i):
                ps = psm.tile([P, CSL], F32, tag="mm", bufs=2, name="ps_v")
                for c in range(NCC):
                    nc.tensor.matmul(
                        ps[:], xt(c, i * P, P), wv_sb[:, c, :],
                        start=(c == 0), stop=(c == NCC - 1),
                    )
                nc.vector.tensor_add(v_sb[:, i, 0, 0:HD], ps[:, 0:HD], bv_bc[:, 0:HD])
                nc.vector.tensor_add(v_sb[:, i, 1, 0:HD], ps[:, HD:CSL], bv_bc[:, HD:CSL])

            def proj_chunk(t):
                # partial projection for token chunk t: [128 tok, C] fp32
                ost = stage.tile([P, C], BF16, tag="ost", bufs=3, name="ost")
                for half in range(2):
                    pp = psm.tile([P, TQ], F32, tag="mm", bufs=2, name="pp")
                    nc.tensor.matmul(
                        pp[:], ot_sb[:, t * P:(t + 1) * P],
                        wp_sb[:, half * TQ:(half + 1) * TQ],
                        start=True, stop=True,
                    )
                    nc.vector.tensor_copy(ost[:, half * TQ:(half + 1) * TQ], pp[:])
                eng = nc.gpsimd if t % 2 == 0 else nc.sync
                eng.dma_start(out_ext[t * P:(t + 1) * P, :], ost[:])

            units = [(b, qc) for b in range(B) for qc in range(NQC)]
            NS = len(units) * NKC  # 128 pipeline steps

            sp_tiles = {}

            def do_S(s):
                # high priority: the exp stream (the bottleneck) is gated on
                # these, so the scheduler must sequence them ahead of any
                # ready PV/proj/qkv backlog on the PE queue
                u, k0 = divmod(s, NKC)
                b, qc = units[u]
                sp = psm.tile([P, 2, TQ], F32, tag="sp", bufs=2, name="sp")
                with tc.high_priority():
                    for h in range(2):
                        nc.tensor.matmul(
                            sp[:, h, :],
                            kt_sb[h * HD:(h + 1) * HD, b * T + k0 * KC: b * T + (k0 + 1) * KC],
                            qt_sb[h * HD:(h + 1) * HD, b * T + qc * TQ: b * T + (qc + 1) * TQ],
                            start=True, stop=True,
                        )
                sp_tiles[s] = sp

            def drain_head(op_t, u, h):
                # op rows 64-127 hold 64 broadcast copies of the softmax
                # denominator (from the duplicated ones columns of the V
                # stationary), so normalization is three DVE ops per head.
                # Emitted per head right after that head's last PV matmul so
                # h0's chain overlaps h1's PV.
                b, qc = units[u]
                base = b * T + qc * TQ
                rb0 = stage.tile([HD, TQ], F32, tag="rb0", bufs=3, name="rb0")
                nc.vector.tensor_copy(rb0[:], op_t[HD:2 * HD, h, :])
                rb = stage.tile([HD, TQ], F32, tag="rb", bufs=3, name="rb")
                nc.vector.reciprocal_approx_fast(rb[:], rb0[:])
                nc.vector.tensor_mul(
                    ot_sb[h * HD:(h + 1) * HD, base:base + TQ],
                    op_t[0:HD, h, :], rb[:],
                )

            # ---- prologue: first K/Q chunks so the exp pipeline starts early
            qk_half(wk_sb, bk_sb, kt_sb, 0, 0)
            qk_half(wk_sb, bk_sb, kt_sb, 0, 1)
            qk_half(wq_sb, bq_sb, qt_sb, 0, 0)
            qk_half(wq_sb, bq_sb, qt_sb, 0, 1)
            do_S(0)
            do_S(1)
            # broadcast the free-axis V bias across partitions (K=1 matmul)
            bb = psm.tile([P, CSL], F32, tag="mm", bufs=2, name="bb")
            nc.tensor.matmul(bb[:], ones1[0:1, 0:P], bv_row[:], start=True, stop=True)
            nc.vector.tensor_copy(bv_bc[:], bb[:])

            # ---- injected tasks: (step, closure), step = earliest emission.
            # Constraints: v_chunk(i) before PV step using chunk i; qk K/Q
            # chunks before the S emission (step 16u-2) that reads them; x
            # region th_i lands ~(7+7*i)us so tasks must not head-of-line
            # block the PE queue on DMA.
            def qk2(s0, w_sb, b_sb, dst, t8):
                return [(s0, lambda: qk_half(w_sb, b_sb, dst, t8, 0)),
                        (s0 + 1, lambda: qk_half(w_sb, b_sb, dst, t8, 1))]

            # v_chunk(i) sits exactly at step i: if its x region is still in
            # flight it only delays its own PV (which needs it anyway),
            # never the S/exp stream. th2/th3-fed tasks are pushed past the
            # measured DMA arrival (~37/44us).
            sched = []
            sched += [(0, lambda: v_chunk(0)), (1, lambda: v_chunk(1))]
            sched += [(i, lambda i=i: v_chunk(i)) for i in range(2, 16)]
            sched += qk2(0, wk_sb, bk_sb, kt_sb, 1)
            sched += qk2(4, wk_sb, bk_sb, kt_sb, 2)
            sched += qk2(8, wk_sb, bk_sb, kt_sb, 3)
            sched += qk2(12, wq_sb, bq_sb, qt_sb, 1)
            sched += qk2(16, wq_sb, bq_sb, qt_sb, 2)
            sched += qk2(18, wq_sb, bq_sb, qt_sb, 3)
            sched += qk2(26, wk_sb, bk_sb, kt_sb, 4)
            sched += qk2(28, wk_sb, bk_sb, kt_sb, 5)
            sched += [(30 + 2 * j, lambda i=16 + j: v_chunk(i)) for j in range(8)]
            sched += qk2(46, wq_sb, bq_sb, qt_sb, 4)
            sched += qk2(48, wk_sb, bk_sb, kt_sb, 6)
            sched += qk2(50, wk_sb, bk_sb, kt_sb, 7)
            sched += [(52 + j, lambda i=24 + j: v_chunk(i)) for j in range(8)]
            sched += qk2(60, wq_sb, bq_sb, qt_sb, 5)
            sched += qk2(80, wq_sb, bq_sb, qt_sb, 6)
            sched += qk2(96, wq_sb, bq_sb, qt_sb, 7)
            sched.sort(key=lambda e: e[0])

            proj_q = []  # dynamic: projection sub-tasks appear after drains
            op_t = None
            si = 0
            for s in range(NS):
                u, k0 = divmod(s, NKC)
                b, qc = units[u]
                if k0 == 0:
                    op_t = psm.tile([P, 2, TQ], F32, tag="op", bufs=1, name="op_t")
                # S then exp lead the emission each step: ACT is the
                # bottleneck engine and an exp emitted behind injected
                # proj/qkv PE work ends up gated on that work completing
                if s + 2 < NS:
                    do_S(s + 2)
                pt = ptp.tile([P, 2, TQ], BF16, tag="pt", bufs=6, name="pt")
                with tc.high_priority():
                    nc.scalar.activation(pt[:], sp_tiles.pop(s)[:], Exp)
                while si < len(sched) and sched[si][0] <= s:
                    sched[si][1]()
                    si += 1
                if proj_q and 4 <= k0 < 12:
                    proj_q.pop(0)()
                for h in range(2):
                    nc.tensor.matmul(
                        op_t[:, h, :],
                        v_sb[:, b * TB + k0, h, :],
                        pt[:, h, :],
                        start=(k0 == 0), stop=(k0 == NKC - 1),
                    )
                    if k0 == NKC - 1:
                        drain_head(op_t, u, h)
                if k0 == NKC - 1:
                    t0 = (b * T + qc * TQ) // P
                    proj_q += [lambda t=t0 + j: proj_chunk(t) for j in range(TQ // P)]
            while proj_q:
                proj_q.pop(0)()

    nc.compile()
    return nc


def _shard_inputs(x, w_qkv, b_qkv, w_proj, b_proj):
    import ml_dtypes

    bf16 = ml_dtypes.bfloat16
    sc = np.float32(HD ** -0.5)
    x2 = np.ascontiguousarray(x.reshape(BT, NCC, P).astype(bf16).transpose(1, 2, 0))

    def wprep(w):  # [C, CSL] -> SBUF layout [P, NCC, CSL], contiguous
        return np.ascontiguousarray(
            w.astype(bf16).reshape(NCC, P, CSL).transpose(1, 0, 2))

    in_maps = []
    for i in range(NCORE):
        h0 = 2 * i
        cs = slice(h0 * HD, h0 * HD + CSL)
        m = {
            "x": x2,
            "wq": wprep(w_qkv[:, 0 * C:1 * C][:, cs] * sc),
            "wk": wprep(w_qkv[:, 1 * C:2 * C][:, cs]),
            "wv": wprep(w_qkv[:, 2 * C:3 * C][:, cs]),
            "wp": np.ascontiguousarray(w_proj[cs, :].astype(bf16)),
            "bq": np.ascontiguousarray((b_qkv[0 * C:1 * C][cs] * sc).reshape(CSL, 1), dtype=np.float32),
            "bk": np.ascontiguousarray(b_qkv[1 * C:2 * C][cs].reshape(CSL, 1), dtype=np.float32),
            "bv": np.ascontiguousarray(b_qkv[2 * C:3 * C][cs].reshape(1, CSL), dtype=np.float32),
        }
        in_maps.append(m)
    return in_maps


def _run(inputs, trace=False):
    from concourse.bass_utils import run_bass_kernel_spmd

    if "nc" not in _CACHE:
        _CACHE["nc"] = _build()
    nc = _CACHE["nc"]
    in_maps = _shard_inputs(
        np.asarray(inputs["x"]), np.asarray(inputs["w_qkv"]), np.asarray(inputs["b_qkv"]),
        np.asarray(inputs["w_proj"]), np.asarray(inputs["b_proj"]))
    res = run_bass_kernel_spmd(nc, in_maps, list(range(NCORE)), trace=trace)
    out = np.zeros((BT, C), dtype=np.float32)
    for i in range(NCORE):
        out += np.asarray(res.results[i]["out"]).astype(np.float32)
    out += np.asarray(inputs["b_proj"], dtype=np.float32)
    return out.reshape(B, T, C), res


def kernel(**inputs) -> np.ndarray:
    out, _ = _run(inputs, trace=False)
    return out
